# revision 10
# baseline (speedup 1.0000x reference)
"""MeanNSE (segment-reduce) Trainium2 kernel — 8 NeuronCores, data-parallel.

v2: PE-array segment reduction. The basin ids are pure index data, so all
index math runs on the host; the device does every FLOP over the 16.7M
float arrays.

Host: stable-sort elements by basin and pack them into per-core [128, C]
(C = 17408 = 34*512) bf16 tiles in "pillar slot" layout: slot s in [0,512)
owns the 4352 elements at positions {(p, g*512 + s) : p<128, g<34}; each
basin is padded (with zeros) to a whole number of slots, so every slot
contains elements of exactly one basin.  Slot sums can then be computed on
the TENSOR engine: a [128,1] ones stationary x [128,512] moving matmul
yields all 512 per-column partition-sums of one 512-col chunk, and the 34
chunk matmuls accumulate in one PSUM bank (per-element has_written logic),
producing Sum over each slot's full pillar.

Device (per core), three stats per slot, engines balanced:
  - TENSOR: 3 stat passes x 34 accumulating matmuls (N=512, ones
    stationary, ~0.42 ns/col) into 3 psum banks; ~40 warm-up matmuls into a
    scratch bank during the initial DMA keep the PE HAM clock at 2.4 GHz.
  - DVE:  d = t - p, d2 = d*d   (tensor_tensor bf16 runs in 2x mode)
  - ACT:  t2 = Square(t)        (activation, 1x)
  - DMA: inputs stream over sync + scalar HW DGE queues and the gpsimd
    software queue, byte-balanced (~3MB each), small tiles first/last.
Outputs are just 3x512 f32 slot sums -> psum drained via DVE/ACT copies
-> one tiny DMA out.

Host: bincount slot sums back to basins (slot->basin map is host data),
combine in float64 with exact integer counts:
  ss_tot = sum_t2 - sum_t^2/count, nse = 1 - ss_res/(ss_tot + 1e-10),
  answer = mean over 671 basins.
"""

import sys

sys.path.insert(0, "/opt/trn_rl_repo")

import numpy as np
import ml_dtypes

import concourse.bacc as bacc
import concourse.mybir as mybir
import concourse.tile as tile
from concourse.bass_utils import run_bass_kernel_spmd

F32 = mybir.dt.float32
BF16 = mybir.dt.bfloat16
BF16_NP = ml_dtypes.bfloat16

N_CORES = 8
N_TOTAL = 16777216
N_BASINS = 671
EPS = 1e-10

P = 128  # partitions
CH = 512  # psum bank width (f32) = matmul N
SUP = 34  # chunks per stat pass
C = SUP * CH  # columns per core (17408)
E_C = P * C  # elements per core (2,228,224)
PILLAR = P * SUP  # elements per slot (4352)
SLOTS = CH  # slots per core (512)
U_TOT = N_CORES * SLOTS  # global slot-units (4096)

# DMA tile plan, in 512-col chunks (sums to SUP=34): small tiles first so
# compute starts early, small tiles last so the tail drains fast.
K_PLAN = [1, 1, 2, 4, 4, 4, 4, 4, 4, 4, 1, 1]
N_WARM = 4  # PE warm-up matmuls bridging preamble -> first data

_AF = mybir.ActivationFunctionType

_cache = {}


def _dma_schedule():
    """Each HW DGE engine queue holds only 4 outstanding DMAs — the 5th
    trigger blocks that engine's sequencer until an earlier DMA completes.
    So scalar (which must run the squares) gets exactly 4 early small tiles;
    sync (otherwise idle) and gpsimd (software DGE, non-blocking triggers)
    alternate the rest so tiles land roughly in tile order."""
    sched = {"scalar": [], "sync": [], "gpsimd": []}
    for t in range(len(K_PLAN)):
        q = "sync" if t % 2 == 0 else "gpsimd"
        sched[q].append(("yt", t))
        sched[q].append(("yp", t))
    return sched


def _build():
    nc = bacc.Bacc()
    yt = nc.declare_dram_parameter("yt", [E_C], BF16, isOutput=False)
    yp = nc.declare_dram_parameter("yp", [E_C], BF16, isOutput=False)
    # out: [sum_t(512) | sum_t2(512) | sum_d2(512)]
    out = nc.declare_dram_parameter("out", [3 * SLOTS], F32, isOutput=True)

    yt2d = yt[:].rearrange("(p c) -> p c", p=P, c=C)
    yp2d = yp[:].rearrange("(p c) -> p c", p=P, c=C)

    sched = _dma_schedule()

    with tile.TileContext(nc) as tc:
        with (
            tc.tile_pool(name="const", bufs=1) as cpool,
            tc.tile_pool(name="io", bufs=1) as io_pool,
            tc.tile_pool(name="dx", bufs=3) as d_pool,
            tc.tile_pool(name="d2x", bufs=4) as d2_pool,
            tc.tile_pool(name="t2x", bufs=4) as t2_pool,
            tc.tile_pool(name="ps", bufs=1, space="PSUM") as psum_pool,
        ):
            ones = cpool.tile([P, 1], BF16, tag="ones")
            warm = cpool.tile([P, CH], BF16, tag="warm")
            outs = cpool.tile([1, 3 * SLOTS], F32, tag="outs")
            nc.vector.memset(ones[:, :], 1.0)
            nc.vector.memset(warm[:, :], 0.0)

            p_t = psum_pool.tile([1, CH], F32, tag="p_t")
            p_t2 = psum_pool.tile([1, CH], F32, tag="p_t2")
            p_d2 = psum_pool.tile([1, CH], F32, tag="p_d2")
            p_w = psum_pool.tile([1, CH], F32, tag="p_w")

            # PE warm-up: keep the HAM activity monitor busy during the
            # initial DMA so real matmuls run at 2.4 GHz.
            for w in range(N_WARM):
                nc.tensor.matmul(
                    p_w[:, :], ones[:, :], warm[:, :],
                    start=(w == 0), stop=(w == N_WARM - 1),
                )

            # stage all input tiles up front across the three DMA queues
            tiles = []
            base = 0
            for t, k in enumerate(K_PLAN):
                tt_ = io_pool.tile([P, k * CH], BF16, tag=f"yt{t}")
                tp_ = io_pool.tile([P, k * CH], BF16, tag=f"yp{t}")
                tiles.append((tt_, tp_, k, base))
                base += k * CH

            def _dst(arr, t):
                return tiles[t][0 if arr == "yt" else 1][:, :]

            def _src(arr, t):
                b, k = tiles[t][3], tiles[t][2]
                src = yt2d if arr == "yt" else yp2d
                return src[:, b : b + k * CH]

            for arr, t in sched["gpsimd"]:
                nc.gpsimd.dma_start(_dst(arr, t), _src(arr, t))
            for arr, t in sched["sync"]:
                nc.sync.dma_start(_dst(arr, t), _src(arr, t))
            for arr, t in sched["scalar"]:
                nc.scalar.dma_start(_dst(arr, t), _src(arr, t))

            # PE consumes the quadratic stats one tile behind the raw-t
            # stream, so it never head-of-line blocks on DVE/ACT results.
            def _mm(psum, src, cg0, k):
                for c in range(k):
                    nc.tensor.matmul(
                        psum[:, :], ones[:, :], src[:, c * CH : (c + 1) * CH],
                        start=(cg0 + c == 0), stop=(cg0 + c == SUP - 1),
                    )

            lagged = []  # (t2_tile, d2_tile, cg0, k) awaiting PE
            cg = 0
            for tt_, tp_, k, base in tiles:
                # raw-t slot sums: only need yt, keep PE fed right away
                _mm(p_t, tt_, cg, k)
                # t^2 on ACT (depends only on yt)
                t2_t = t2_pool.tile([P, k * CH], BF16, tag="t2")
                nc.scalar.activation(t2_t[:, :], tt_[:, :], _AF.Square)
                # (t-p)^2 on DVE (both tensor_tensor ops run 2x in bf16)
                d_t = d_pool.tile([P, k * CH], BF16, tag="d")
                nc.vector.tensor_sub(d_t[:, :], tt_[:, :], tp_[:, :])
                d2_t = d2_pool.tile([P, k * CH], BF16, tag="d2")
                nc.vector.tensor_mul(d2_t[:, :], d_t[:, :], d_t[:, :])
                lagged.append((t2_t, d2_t, cg, k))
                if len(lagged) > 1:
                    lt2, ld2, lcg, lk = lagged.pop(0)
                    _mm(p_t2, lt2, lcg, lk)
                    _mm(p_d2, ld2, lcg, lk)
                cg += k

            # p_t is complete: drain it on ACT while PE flushes the last tile
            nc.scalar.activation(outs[:, 0:CH], p_t[:, :], _AF.Copy)
            lt2, ld2, lcg, lk = lagged.pop(0)
            _mm(p_t2, lt2, lcg, lk)  # t2 ready before d2: flush it first
            _mm(p_d2, ld2, lcg, lk)
            # drain remaining psum banks on two engines in parallel
            nc.vector.tensor_copy(outs[:, CH : 2 * CH], p_t2[:, :])
            nc.scalar.activation(outs[:, 2 * CH : 3 * CH], p_d2[:, :], _AF.Copy)

            # out DMA on the scalar engine: its HW queue carries no inputs,
            # and the trigger naturally follows the p_d2 drain in its FIFO
            nc.scalar.dma_start(
                out[:].rearrange("(p x) -> p x", p=1, x=3 * SLOTS),
                outs[:, :],
            )
    nc.compile()
    return nc


def _get_nc():
    if "nc" not in _cache:
        _cache["nc"] = _build()
    return _cache["nc"]


def _prepare(y_pred, y_true, basin):
    """Host-side index math: sort by basin, pack into pillar-slot layout."""
    y_pred = np.asarray(y_pred, dtype=np.float32)
    y_true = np.asarray(y_true, dtype=np.float32)
    b = np.asarray(basin).astype(np.int32)
    n = b.shape[0]

    counts = np.bincount(b, minlength=N_BASINS)
    m = (counts + PILLAR - 1) // PILLAR  # slots per basin
    u_tot = int(m.sum())
    assert u_tot <= U_TOT, (u_tot, U_TOT)
    base_u = np.zeros(N_BASINS + 1, np.int64)
    np.cumsum(m, out=base_u[1:])

    order = np.argsort(b, kind="stable")
    seg_start = np.zeros(N_BASINS, np.int64)
    np.cumsum(counts[:-1], out=seg_start[1:])
    bs = b[order]
    i_local = np.arange(n, dtype=np.int64) - seg_start[bs]
    su = base_u[bs] + i_local // PILLAR  # global slot-unit
    j = i_local % PILLAR
    p = j // SUP
    g = j % SUP
    core = su // SLOTS
    s = su % SLOTS
    dst = core * E_C + p * C + g * CH + s

    yt_pad = np.zeros(N_CORES * E_C, dtype=BF16_NP)
    yp_pad = np.zeros(N_CORES * E_C, dtype=BF16_NP)
    yt_pad[dst] = y_true[order].astype(BF16_NP)
    yp_pad[dst] = y_pred[order].astype(BF16_NP)
    yt_pad = yt_pad.reshape(N_CORES, E_C)
    yp_pad = yp_pad.reshape(N_CORES, E_C)

    in_maps = [{"yt": yt_pad[c], "yp": yp_pad[c]} for c in range(N_CORES)]

    # basin of every global slot-unit (pad units -> N_BASINS, dropped later)
    slot_basin = np.full(U_TOT, N_BASINS, np.int64)
    slot_basin[:u_tot] = np.repeat(np.arange(N_BASINS), m)
    return in_maps, (counts, slot_basin)


def _finish(results, ctx):
    counts, slot_basin = ctx
    sums = np.empty((3, U_TOT), np.float64)
    for c in range(N_CORES):
        arr = np.asarray(results[c]["out"], np.float64).reshape(3, SLOTS)
        sums[:, c * SLOTS : (c + 1) * SLOTS] = arr
    s_t = np.bincount(slot_basin, weights=sums[0], minlength=N_BASINS + 1)[:N_BASINS]
    s_t2 = np.bincount(slot_basin, weights=sums[1], minlength=N_BASINS + 1)[:N_BASINS]
    s_d2 = np.bincount(slot_basin, weights=sums[2], minlength=N_BASINS + 1)[:N_BASINS]
    cnt = counts.astype(np.float64)
    ss_tot = s_t2 - s_t * s_t / cnt
    nse = 1.0 - s_d2 / (ss_tot + EPS)
    return np.float32(nse.mean())


def kernel(y_pred, y_true, basin):
    in_maps, ctx = _prepare(y_pred, y_true, basin)
    res = run_bass_kernel_spmd(_get_nc(), in_maps, list(range(N_CORES)))
    return _finish(res.results, ctx)


# revision 11
# speedup vs baseline: 1.0433x; 1.0433x over previous
"""MeanNSE (segment-reduce) Trainium2 kernel — 8 NeuronCores, data-parallel.

v2: PE-array segment reduction. The basin ids are pure index data, so all
index math runs on the host; the device does every FLOP over the 16.7M
float arrays.

Host: stable-sort elements by basin and pack them into per-core [128, C]
(C = 17408 = 34*512) bf16 tiles in "pillar slot" layout: slot s in [0,512)
owns the 4352 elements at positions {(p, g*512 + s) : p<128, g<34}; each
basin is padded (with zeros) to a whole number of slots, so every slot
contains elements of exactly one basin.  Slot sums can then be computed on
the TENSOR engine: a [128,1] ones stationary x [128,512] moving matmul
yields all 512 per-column partition-sums of one 512-col chunk, and the 34
chunk matmuls accumulate in one PSUM bank (per-element has_written logic),
producing Sum over each slot's full pillar.

Device (per core), three stats per slot, engines balanced:
  - TENSOR: 3 stat passes x 34 accumulating matmuls (N=512, ones
    stationary, ~0.42 ns/col) into 3 psum banks; ~40 warm-up matmuls into a
    scratch bank during the initial DMA keep the PE HAM clock at 2.4 GHz.
  - DVE:  d = t - p, d2 = d*d   (tensor_tensor bf16 runs in 2x mode)
  - ACT:  t2 = Square(t)        (activation, 1x)
  - DMA: inputs stream over sync + scalar HW DGE queues and the gpsimd
    software queue, byte-balanced (~3MB each), small tiles first/last.
Outputs are just 3x512 f32 slot sums -> psum drained via DVE/ACT copies
-> one tiny DMA out.

Host: bincount slot sums back to basins (slot->basin map is host data),
combine in float64 with exact integer counts:
  ss_tot = sum_t2 - sum_t^2/count, nse = 1 - ss_res/(ss_tot + 1e-10),
  answer = mean over 671 basins.
"""

import sys

sys.path.insert(0, "/opt/trn_rl_repo")

import numpy as np
import ml_dtypes

import concourse.bacc as bacc
import concourse.mybir as mybir
import concourse.tile as tile
from concourse.bass_utils import run_bass_kernel_spmd

F32 = mybir.dt.float32
BF16 = mybir.dt.bfloat16
BF16_NP = ml_dtypes.bfloat16

N_CORES = 8
N_TOTAL = 16777216
N_BASINS = 671
EPS = 1e-10

P = 128  # partitions
CH = 512  # psum bank width (f32) = matmul N
SUP = 34  # chunks per stat pass
C = SUP * CH  # columns per core (17408)
E_C = P * C  # elements per core (2,228,224)
PILLAR = P * SUP  # elements per slot (4352)
SLOTS = CH  # slots per core (512)
U_TOT = N_CORES * SLOTS  # global slot-units (4096)

# DMA tile plan, in 512-col chunks (sums to SUP=34): small tiles first so
# compute starts early, small tiles last so the tail drains fast.
K_PLAN = [1, 1, 2, 4, 4, 4, 4, 4, 4, 4, 1, 1]
N_WARM = 4  # PE warm-up matmuls bridging preamble -> first data

_AF = mybir.ActivationFunctionType

_cache = {}


def _dma_schedule():
    """Each HW DGE engine queue holds only 4 outstanding DMAs — the 5th
    trigger blocks that engine's sequencer until an earlier DMA completes.
    So scalar (which must run the squares) gets exactly 4 early small tiles;
    sync (otherwise idle) and gpsimd (software DGE, non-blocking triggers)
    alternate the rest so tiles land roughly in tile order."""
    sched = {"scalar": [], "sync": [], "gpsimd": []}
    order = []
    for t in range(len(K_PLAN)):
        order.append(("yt", t))
        order.append(("yp", t))
    rr = ["sync", "gpsimd", "scalar"]
    for i, pair in enumerate(order):
        q = rr[i % 3]
        if q == "scalar" and len(sched["scalar"]) >= 4:
            q = rr[i % 2]  # alternate sync/gpsimd once scalar has its 4
        sched[q].append(pair)
    return sched


def _build():
    nc = bacc.Bacc()
    yt = nc.declare_dram_parameter("yt", [E_C], BF16, isOutput=False)
    yp = nc.declare_dram_parameter("yp", [E_C], BF16, isOutput=False)
    # out: [sum_t(512) | sum_t2(512) | sum_d2(512)]
    out = nc.declare_dram_parameter("out", [3 * SLOTS], F32, isOutput=True)

    yt2d = yt[:].rearrange("(p c) -> p c", p=P, c=C)
    yp2d = yp[:].rearrange("(p c) -> p c", p=P, c=C)

    sched = _dma_schedule()

    with tile.TileContext(nc) as tc:
        with (
            tc.tile_pool(name="const", bufs=1) as cpool,
            tc.tile_pool(name="io", bufs=1) as io_pool,
            tc.tile_pool(name="dx", bufs=3) as d_pool,
            tc.tile_pool(name="d2x", bufs=4) as d2_pool,
            tc.tile_pool(name="t2x", bufs=4) as t2_pool,
            tc.tile_pool(name="ps", bufs=1, space="PSUM") as psum_pool,
        ):
            ones = cpool.tile([P, 1], BF16, tag="ones")
            warm = cpool.tile([P, CH], BF16, tag="warm")
            outs = cpool.tile([1, 3 * SLOTS], F32, tag="outs")
            nc.vector.memset(ones[:, :], 1.0)
            nc.vector.memset(warm[:, :], 0.0)

            p_t = psum_pool.tile([1, CH], F32, tag="p_t")
            p_t2 = psum_pool.tile([1, CH], F32, tag="p_t2")
            p_d2 = psum_pool.tile([1, CH], F32, tag="p_d2")
            p_w = psum_pool.tile([1, CH], F32, tag="p_w")

            # PE warm-up: keep the HAM activity monitor busy during the
            # initial DMA so real matmuls run at 2.4 GHz.
            for w in range(N_WARM):
                nc.tensor.matmul(
                    p_w[:, :], ones[:, :], warm[:, :],
                    start=(w == 0), stop=(w == N_WARM - 1),
                )

            # stage all input tiles up front across the three DMA queues
            tiles = []
            base = 0
            for t, k in enumerate(K_PLAN):
                tt_ = io_pool.tile([P, k * CH], BF16, tag=f"yt{t}")
                tp_ = io_pool.tile([P, k * CH], BF16, tag=f"yp{t}")
                tiles.append((tt_, tp_, k, base))
                base += k * CH

            def _dst(arr, t):
                return tiles[t][0 if arr == "yt" else 1][:, :]

            def _src(arr, t):
                b, k = tiles[t][3], tiles[t][2]
                src = yt2d if arr == "yt" else yp2d
                return src[:, b : b + k * CH]

            for arr, t in sched["gpsimd"]:
                nc.gpsimd.dma_start(_dst(arr, t), _src(arr, t))
            for arr, t in sched["sync"]:
                nc.sync.dma_start(_dst(arr, t), _src(arr, t))
            for arr, t in sched["scalar"]:
                nc.scalar.dma_start(_dst(arr, t), _src(arr, t))

            # PE consumes the quadratic stats one tile behind the raw-t
            # stream, so it never head-of-line blocks on DVE/ACT results.
            def _mm(psum, src, cg0, k):
                for c in range(k):
                    nc.tensor.matmul(
                        psum[:, :], ones[:, :], src[:, c * CH : (c + 1) * CH],
                        start=(cg0 + c == 0), stop=(cg0 + c == SUP - 1),
                    )

            lagged = []  # (t2_tile, d2_tile, cg0, k) awaiting PE
            cg = 0
            for tt_, tp_, k, base in tiles:
                # raw-t slot sums: only need yt, keep PE fed right away
                _mm(p_t, tt_, cg, k)
                # t^2 on ACT (depends only on yt)
                t2_t = t2_pool.tile([P, k * CH], BF16, tag="t2")
                nc.scalar.activation(t2_t[:, :], tt_[:, :], _AF.Square)
                # (t-p)^2 on DVE (both tensor_tensor ops run 2x in bf16)
                d_t = d_pool.tile([P, k * CH], BF16, tag="d")
                nc.vector.tensor_sub(d_t[:, :], tt_[:, :], tp_[:, :])
                d2_t = d2_pool.tile([P, k * CH], BF16, tag="d2")
                nc.vector.tensor_mul(d2_t[:, :], d_t[:, :], d_t[:, :])
                lagged.append((t2_t, d2_t, cg, k))
                if len(lagged) > 1:
                    lt2, ld2, lcg, lk = lagged.pop(0)
                    _mm(p_t2, lt2, lcg, lk)
                    _mm(p_d2, ld2, lcg, lk)
                cg += k

            # p_t is complete: drain it on ACT while PE flushes the last tile
            nc.scalar.activation(outs[:, 0:CH], p_t[:, :], _AF.Copy)
            lt2, ld2, lcg, lk = lagged.pop(0)
            _mm(p_t2, lt2, lcg, lk)  # t2 ready before d2: flush it first
            _mm(p_d2, ld2, lcg, lk)
            # drain remaining psum banks on two engines in parallel
            nc.vector.tensor_copy(outs[:, CH : 2 * CH], p_t2[:, :])
            nc.scalar.activation(outs[:, 2 * CH : 3 * CH], p_d2[:, :], _AF.Copy)

            # out DMA on the scalar engine: its HW queue carries no inputs,
            # and the trigger naturally follows the p_d2 drain in its FIFO
            nc.scalar.dma_start(
                out[:].rearrange("(p x) -> p x", p=1, x=3 * SLOTS),
                outs[:, :],
            )
    nc.compile()
    return nc


def _get_nc():
    if "nc" not in _cache:
        _cache["nc"] = _build()
    return _cache["nc"]


def _prepare(y_pred, y_true, basin):
    """Host-side index math: sort by basin, pack into pillar-slot layout."""
    y_pred = np.asarray(y_pred, dtype=np.float32)
    y_true = np.asarray(y_true, dtype=np.float32)
    b = np.asarray(basin).astype(np.int32)
    n = b.shape[0]

    counts = np.bincount(b, minlength=N_BASINS)
    m = (counts + PILLAR - 1) // PILLAR  # slots per basin
    u_tot = int(m.sum())
    assert u_tot <= U_TOT, (u_tot, U_TOT)
    base_u = np.zeros(N_BASINS + 1, np.int64)
    np.cumsum(m, out=base_u[1:])

    order = np.argsort(b, kind="stable")
    seg_start = np.zeros(N_BASINS, np.int64)
    np.cumsum(counts[:-1], out=seg_start[1:])
    bs = b[order]
    i_local = np.arange(n, dtype=np.int64) - seg_start[bs]
    su = base_u[bs] + i_local // PILLAR  # global slot-unit
    j = i_local % PILLAR
    p = j // SUP
    g = j % SUP
    core = su // SLOTS
    s = su % SLOTS
    dst = core * E_C + p * C + g * CH + s

    yt_pad = np.zeros(N_CORES * E_C, dtype=BF16_NP)
    yp_pad = np.zeros(N_CORES * E_C, dtype=BF16_NP)
    yt_pad[dst] = y_true[order].astype(BF16_NP)
    yp_pad[dst] = y_pred[order].astype(BF16_NP)
    yt_pad = yt_pad.reshape(N_CORES, E_C)
    yp_pad = yp_pad.reshape(N_CORES, E_C)

    in_maps = [{"yt": yt_pad[c], "yp": yp_pad[c]} for c in range(N_CORES)]

    # basin of every global slot-unit (pad units -> N_BASINS, dropped later)
    slot_basin = np.full(U_TOT, N_BASINS, np.int64)
    slot_basin[:u_tot] = np.repeat(np.arange(N_BASINS), m)
    return in_maps, (counts, slot_basin)


def _finish(results, ctx):
    counts, slot_basin = ctx
    sums = np.empty((3, U_TOT), np.float64)
    for c in range(N_CORES):
        arr = np.asarray(results[c]["out"], np.float64).reshape(3, SLOTS)
        sums[:, c * SLOTS : (c + 1) * SLOTS] = arr
    s_t = np.bincount(slot_basin, weights=sums[0], minlength=N_BASINS + 1)[:N_BASINS]
    s_t2 = np.bincount(slot_basin, weights=sums[1], minlength=N_BASINS + 1)[:N_BASINS]
    s_d2 = np.bincount(slot_basin, weights=sums[2], minlength=N_BASINS + 1)[:N_BASINS]
    cnt = counts.astype(np.float64)
    ss_tot = s_t2 - s_t * s_t / cnt
    nse = 1.0 - s_d2 / (ss_tot + EPS)
    return np.float32(nse.mean())


def kernel(y_pred, y_true, basin):
    in_maps, ctx = _prepare(y_pred, y_true, basin)
    res = run_bass_kernel_spmd(_get_nc(), in_maps, list(range(N_CORES)))
    return _finish(res.results, ctx)


# revision 12
# speedup vs baseline: 1.1299x; 1.0830x over previous
"""MeanNSE (segment-reduce) Trainium2 kernel — 8 NeuronCores, data-parallel.

v8: PE-array segment reduction, zero-padding layout. The basin ids are pure
index data, so all index math runs on the host; the device does every FLOP
over the 16.7M-element float arrays.

Host: stable-sort elements by basin. 16,777,216 = 8 cores x 128 x 16384
exactly, so the sorted stream packs into per-core [128, C=16384] bf16 tiles
with NO padding: sorted element e lands at core e>>21, partition (e%2^21
%4096)//32, column g*512+slot with g=(e%4096)%32, slot=(e%2^21)//4096.
Device "slot" sums (512 per core, accumulated over the 32 chunk matmuls)
are then exactly the sums of contiguous 4096-element ranges of the sorted
stream.  Basin boundaries cut through at most one slot each; the host
reconstructs exact per-basin sums from the device slot sums plus ~671 tiny
boundary-fragment sums it computes itself (replicating the device's bf16
elementwise arithmetic on those fragments only).

Device (per core), three stats per slot:
  - TENSOR: 3 stat passes x 32 accumulating matmuls (N=512, [128,1] ones
    stationary, 512 col-sums per matmul) into 3 psum banks; 4 warm-up
    matmuls bridge the DMA ramp so real matmuls run at the 2.4 GHz p-state.
  - DVE:  d = t - p, d2 = d*d   (tensor_tensor bf16 runs in 2x mode)
  - ACT:  t2 = Square(t)        (activation, 1x) + psum drains
  - DMA: 16 graded pieces per array (small first and last); scalar's HW
    queue carries only the first 2 pieces of each array (its 4-deep DGE
    ring never blocks the squares), sync/gpsimd alternate the rest so
    pieces land in order; quadratic-stat matmuls run one block behind the
    raw-t stream.  Sum_t drains early; its out-DMA half overlaps the tail.

Host: prefix device slot sums + boundary fragments -> exact per-basin
sums in float64, exact integer counts:
  ss_tot = sum_t2 - sum_t^2/count, nse = 1 - ss_res/(ss_tot + 1e-10),
  answer = mean over 671 basins.
"""

import sys

sys.path.insert(0, "/opt/trn_rl_repo")

import numpy as np
import ml_dtypes

import concourse.bacc as bacc
import concourse.mybir as mybir
import concourse.tile as tile
from concourse.bass_utils import run_bass_kernel_spmd

F32 = mybir.dt.float32
BF16 = mybir.dt.bfloat16
BF16_NP = ml_dtypes.bfloat16

N_CORES = 8
N_TOTAL = 16777216
N_BASINS = 671
EPS = 1e-10

P = 128  # partitions
CH = 512  # psum bank width (f32) = matmul N = slots per core
SUP = 32  # chunks per stat pass
C = SUP * CH  # columns per core (16384)
E_C = P * C  # elements per core (2,097,152 = 2^21)
UNIT = P * SUP  # elements per slot (4096)
U_TOT = N_TOTAL // UNIT  # global slot-units (4096)

# DMA piece plan == compute block plan, in 512-col chunks (sums to SUP=32):
# small pieces first (compute starts early) and last (short tail).
K_PLAN = [1, 1, 2, 2, 2, 2, 3, 3, 3, 3, 3, 2, 2, 1, 1, 1]
N_WARM = 4

_AF = mybir.ActivationFunctionType

_cache = {}


def _dma_schedule():
    """scalar's 4-deep HW DGE ring gets exactly the first two pieces of each
    array (so its sequencer never blocks before the squares); sync and
    gpsimd alternate the rest, each piece's yt/yp on different queues, so
    pieces stream in near-perfect order."""
    sched = {"scalar": [], "sync": [], "gpsimd": []}
    for t in range(len(K_PLAN)):
        if t < 2:
            sched["scalar"] += [("yt", t), ("yp", t)]
        else:
            a, b = ("sync", "gpsimd") if t % 2 == 0 else ("gpsimd", "sync")
            sched[a].append(("yt", t))
            sched[b].append(("yp", t))
    return sched


def _build():
    nc = bacc.Bacc()
    yt = nc.declare_dram_parameter("yt", [E_C], BF16, isOutput=False)
    yp = nc.declare_dram_parameter("yp", [E_C], BF16, isOutput=False)
    # out: [sum_t(512) | sum_t2(512) | sum_d2(512)]
    out = nc.declare_dram_parameter("out", [3 * CH], F32, isOutput=True)

    yt2d = yt[:].rearrange("(p c) -> p c", p=P, c=C)
    yp2d = yp[:].rearrange("(p c) -> p c", p=P, c=C)
    out2d = out[:].rearrange("(p x) -> p x", p=1, x=3 * CH)

    sched = _dma_schedule()

    with tile.TileContext(nc) as tc:
        with (
            tc.tile_pool(name="const", bufs=1) as cpool,
            tc.tile_pool(name="io", bufs=1) as io_pool,
            tc.tile_pool(name="dx", bufs=3) as d_pool,
            tc.tile_pool(name="d2x", bufs=4) as d2_pool,
            tc.tile_pool(name="t2x", bufs=4) as t2_pool,
            tc.tile_pool(name="ps", bufs=1, space="PSUM") as psum_pool,
        ):
            ones = cpool.tile([P, 1], BF16, tag="ones")
            warm = cpool.tile([P, CH], BF16, tag="warm")
            outs = cpool.tile([1, 3 * CH], F32, tag="outs")
            nc.vector.memset(ones[:, :], 1.0)
            nc.vector.memset(warm[:, :], 0.0)

            p_t = psum_pool.tile([1, CH], F32, tag="p_t")
            p_t2 = psum_pool.tile([1, CH], F32, tag="p_t2")
            p_d2 = psum_pool.tile([1, CH], F32, tag="p_d2")
            p_w = psum_pool.tile([1, CH], F32, tag="p_w")

            # PE warm-up: engage the p-state ramp before real data arrives
            for w in range(N_WARM):
                nc.tensor.matmul(
                    p_w[:, :], ones[:, :], warm[:, :],
                    start=(w == 0), stop=(w == N_WARM - 1),
                )

            # whole-array IO tiles; DMA pieces land in slices (Tile tracks
            # slice-level dependencies)
            T = io_pool.tile([P, C], BF16, tag="T")
            Q = io_pool.tile([P, C], BF16, tag="Q")
            bounds = []
            b0 = 0
            for k in K_PLAN:
                bounds.append((b0 * CH, (b0 + k) * CH))
                b0 += k

            for q in ("gpsimd", "sync", "scalar"):
                eng = getattr(nc, q)
                for arr, t in sched[q]:
                    a, b = bounds[t]
                    src = (yt2d if arr == "yt" else yp2d)[:, a:b]
                    dst = (T if arr == "yt" else Q)[:, a:b]
                    eng.dma_start(dst, src)

            def _mm(psum, src, cg0, k):
                for c in range(k):
                    nc.tensor.matmul(
                        psum[:, :], ones[:, :], src[:, c * CH : (c + 1) * CH],
                        start=(cg0 + c == 0), stop=(cg0 + c == SUP - 1),
                    )

            # PE consumes the quadratic stats one block behind the raw-t
            # stream, so it never head-of-line blocks on DVE/ACT results.
            lagged = []
            cg = 0
            for k in K_PLAN:
                sl = slice(cg * CH, (cg + k) * CH)
                _mm(p_t, T[:, sl], cg, k)
                t2_t = t2_pool.tile([P, k * CH], BF16, tag="t2")
                nc.scalar.activation(t2_t[:, :], T[:, sl], _AF.Square)
                d_t = d_pool.tile([P, k * CH], BF16, tag="d")
                nc.vector.tensor_sub(d_t[:, :], T[:, sl], Q[:, sl])
                d2_t = d2_pool.tile([P, k * CH], BF16, tag="d2")
                nc.vector.tensor_mul(d2_t[:, :], d_t[:, :], d_t[:, :])
                lagged.append((t2_t, d2_t, cg, k))
                if len(lagged) > 1:
                    lt2, ld2, lcg, lk = lagged.pop(0)
                    _mm(p_t2, lt2[:, :], lcg, lk)
                    _mm(p_d2, ld2[:, :], lcg, lk)
                cg += k

            # p_t is complete: drain + ship it while PE flushes the last block
            nc.scalar.activation(outs[:, 0:CH], p_t[:, :], _AF.Copy)
            nc.sync.dma_start(out2d[:, 0:CH], outs[:, 0:CH])
            lt2, ld2, lcg, lk = lagged.pop(0)
            _mm(p_t2, lt2[:, :], lcg, lk)
            _mm(p_d2, ld2[:, :], lcg, lk)
            nc.vector.tensor_copy(outs[:, CH : 2 * CH], p_t2[:, :])
            nc.scalar.activation(outs[:, 2 * CH : 3 * CH], p_d2[:, :], _AF.Copy)
            nc.sync.dma_start(out2d[:, CH : 3 * CH], outs[:, CH : 3 * CH])
    nc.compile()
    return nc


def _get_nc():
    if "nc" not in _cache:
        _cache["nc"] = _build()
    return _cache["nc"]


def _prepare(y_pred, y_true, basin):
    """Host-side index math: sort by basin, pack the contiguous sorted
    stream into the per-core [128, C] layout (a pure permutation, no pads),
    and precompute the boundary-fragment sums."""
    y_pred = np.asarray(y_pred, dtype=np.float32)
    y_true = np.asarray(y_true, dtype=np.float32)
    b = np.asarray(basin).astype(np.int32)
    n = b.shape[0]
    assert n == N_TOTAL

    counts = np.bincount(b, minlength=N_BASINS)
    starts = np.zeros(N_BASINS + 1, np.int64)
    np.cumsum(counts, out=starts[1:])

    order = np.argsort(b, kind="stable")
    ts = y_true[order].astype(BF16_NP)  # device-exact bf16 stream
    ps = y_pred[order].astype(BF16_NP)

    e = np.arange(n, dtype=np.int64)
    j = e & (E_C - 1)
    r = j & (UNIT - 1)
    dst = (e >> 21) * E_C + (r >> 5) * C + (r & 31) * CH + (j >> 12)

    yt_pad = np.empty(N_CORES * E_C, dtype=BF16_NP)
    yp_pad = np.empty(N_CORES * E_C, dtype=BF16_NP)
    yt_pad[dst] = ts
    yp_pad[dst] = ps
    yt_pad = yt_pad.reshape(N_CORES, E_C)
    yp_pad = yp_pad.reshape(N_CORES, E_C)
    in_maps = [{"yt": yt_pad[c], "yp": yp_pad[c]} for c in range(N_CORES)]

    # boundary fragments: P_x[b] = sum over [unit_start, o_b) of each stat,
    # replicating the device's bf16 elementwise arithmetic exactly
    o = starts  # 672 boundaries, o[0]=0, o[-1]=n
    frag_lo = (o >> 12) << 12
    seg_len = (o - frag_lo).astype(np.int64)  # 0..4095
    P_sums = np.zeros((3, N_BASINS + 1), np.float64)
    nz = np.nonzero(seg_len)[0]
    if nz.size:
        idx = np.concatenate([np.arange(frag_lo[i], o[i]) for i in nz])
        seg = np.repeat(np.arange(nz.size), seg_len[nz])
        tq = ts[idx].astype(np.float32)
        pq = ps[idx].astype(np.float32)
        t2q = (tq * tq).astype(BF16_NP).astype(np.float64)
        dq = (tq - pq).astype(BF16_NP).astype(np.float32)
        d2q = (dq * dq).astype(BF16_NP).astype(np.float64)
        m = nz.size
        P_sums[0, nz] = np.bincount(seg, weights=tq.astype(np.float64), minlength=m)
        P_sums[1, nz] = np.bincount(seg, weights=t2q, minlength=m)
        P_sums[2, nz] = np.bincount(seg, weights=d2q, minlength=m)
    return in_maps, (counts, o, P_sums)


def _finish(results, ctx):
    counts, o, P_sums = ctx
    U = np.empty((3, U_TOT), np.float64)
    for c in range(N_CORES):
        arr = np.asarray(results[c]["out"], np.float64).reshape(3, CH)
        U[:, c * CH : (c + 1) * CH] = arr
    D = np.zeros((3, U_TOT + 1), np.float64)
    np.cumsum(U, axis=1, out=D[:, 1:])
    # prefix totals at each basin boundary, then per-basin diffs
    Tb = D[:, o >> 12] + P_sums  # [3, 672]
    s_t, s_t2, s_d2 = np.diff(Tb, axis=1)
    cnt = counts.astype(np.float64)
    ss_tot = s_t2 - s_t * s_t / cnt
    nse = 1.0 - s_d2 / (ss_tot + EPS)
    return np.float32(nse.mean())


def kernel(y_pred, y_true, basin):
    in_maps, ctx = _prepare(y_pred, y_true, basin)
    res = run_bass_kernel_spmd(_get_nc(), in_maps, list(range(N_CORES)))
    return _finish(res.results, ctx)
